# revision 20
# baseline (speedup 1.0000x reference)
"""Trainium2 Bass kernel for nn_CachePredictor (moe_routing).

Computation (see reference):
    x = relu(feature @ W_up.T + b_up)                      [B, 512]
    t_out = sigmoid(einsum('bf,bgf', x, W_table[tids]) + b_table[tids]) * tmask
    i_out = sigmoid(einsum('bf,bgf', x, W_index[iids]) + b_index[iids]) * imask
    out = stack([t_out, i_out])                            [2, B, 256]

Strategy: expert sharding. Per-sample gather of expert weights would move
~4 GB of HBM traffic; grouping samples by expert reads each expert matrix
exactly once. Each of the 8 cores owns 8 table experts and 16 index
experts and processes only the samples routed to its experts. The host
computes routing metadata (sample->expert grouping, capacity padding) and
arranges per-core inputs; all FLOPs (both matmul stages, relu, sigmoid,
bias adds) run on device.

Device program per core (single SPMD program; capacity-padded so all cores
share identical shapes):
  stage 1:  xT[512, nCols] = relu(W_upT.T @ featT + b_up)   PE + DVE(bias)
  stage 2:  per expert PAIR: out[rows, 2*256] = sigmoid((xT_seg.T @
            [W_e0|W_e1].T * 32 + 32*b) / 32)

Numerics: expert weights are fp8 e3m4 scaled by 32 (their dynamic range is
tiny: N(0, 1/512)); the PE allows mixed-dtype matmuls, so x stays bf16 as
the stationary operand while fp8 weights stream as the moving operand.
PSUM accumulates fp32; the 1/32 descale is folded into the sigmoid
ACTIVATE's scale. Outputs are written fp16 (sigmoid range [0,1]).
Measured end-to-end max rel err ~1.4e-2 (threshold 2e-2).

Perf structure (why it looks like this):
- fp8 halves the dominant HBM stream (weights: 3.15 MB/core) vs bf16.
  The PE moving-operand ingest (128 elem/cycle @ 2.4 GHz) then roughly
  matches the DMA rate, so stage 2 streams near both ceilings at once.
- ALL weight-chunk DMAs are emitted before the compute loop: each HWDGE
  ring's FIFO is [fa half, 6 weight half-chunks], dispatched back-to-back
  from kernel start so chunks arrive ahead of PE consumption (no
  just-in-time stalls). Each 0.5 MiB chunk is split into two
  fully-contiguous half-DMAs, one per ring, so chunks complete in
  consumption order at the combined ring rate.
- Experts are processed in PAIRS sharing one column segment: every sample
  is multiplied against BOTH experts' weights in one N=512 moving pass and
  the host keeps the valid half. Redundant FLOPs are free; PE time is set
  by moving columns, which equal the weight volume either way.
- Consecutive matmuls alternate PSUM banks; the pair-bias add (scaled x32)
  is a K=1 fp32r ones-outer-product matmul issued FIRST so it initializes
  the full PSUM zero region.
- Small inputs (W_up, biases) and bulk outputs ride the SWDGE queue; the
  final chunk's outputs use the HWDGE rings so the kernel tail is not a
  Q7 drain. Both pairs of a chunk share one fp16 output tile so there is
  a single output DMA per chunk.
- A few dummy warmup matmuls run during the initial DMA-latency window to
  pre-ramp the HAM clock gate (cold PE runs at 1.2 GHz, warm at 2.4) so
  stage 1/2 run mostly warm.
- Activations stay transposed ([feature, sample]) end to end.

Masked-off samples are never routed (reference zeroes them); the host
scatters computed rows back and leaves the rest zero.
"""

import ml_dtypes
import numpy as np

_N_CORES = 8
_F = 256        # feature dim
_HID = 512      # up-projection width
_G = 256        # buckets
_N_TABLES = 64
_N_INDEXES = 128
_TPC = _N_TABLES // _N_CORES    # table experts per core
_IPC = _N_INDEXES // _N_CORES   # index experts per core
_CPE = 4                        # experts per weight chunk (0.5 MiB fp8)
_WSCALE = 32.0                  # fp8 e3m4 weight scale

_nc_cache = {}

# Set by a test harness to capture HW profiles; harmless when unused.
TRACE = False
LAST_RESULTS = None


def _build(Cpt, Cpi):
    """Build + compile the SPMD program for per-PAIR capacities (Cpt, Cpi)."""
    from concourse import bacc
    import concourse.tile as tile
    import concourse.mybir as mybir

    F32 = mybir.dt.float32
    F32R = mybir.dt.float32r
    BF16 = mybir.dt.bfloat16
    FP8E3 = mybir.dt.float8e3
    FP16 = mybir.dt.float16
    AF = mybir.ActivationFunctionType

    TP = _TPC // 2   # table pairs per core (4)
    IP = _IPC // 2   # index pairs per core (8)
    NTcols = TP * Cpt
    NIcols = IP * Cpi
    TCH = _TPC // _CPE   # table weight chunks (2)
    ICH = _IPC // _CPE   # index weight chunks (4)

    nc = bacc.Bacc(
        "TRN2",
        target_bir_lowering=False,
        debug=False,
        enable_asserts=False,
        num_devices=_N_CORES,
    )
    NA = NTcols + NIcols
    # stage-1 inputs packed into tensors with long rows so the leading ring
    # DMAs run at full rate: per K-half [featT_table | W_upT], then the
    # index features (needed ~3us later than the table part)
    s1a = nc.dram_tensor("s1a", [128, 2, NTcols + _HID], BF16, kind="ExternalInput").ap()
    s1b = nc.dram_tensor("s1b", [128, 2, NIcols], BF16, kind="ExternalInput").ap()
    # host-packed, partition-major: [chunk, p, e_local*1024 + c*256 + g]
    wt = nc.dram_tensor("wt", [TCH, 128, _CPE * 4 * _G], FP8E3, kind="ExternalInput").ap()
    wi = nc.dram_tensor("wi", [ICH, 128, _CPE * 4 * _G], FP8E3, kind="ExternalInput").ap()
    bt = nc.dram_tensor("bt", [1, _TPC * _G], BF16, kind="ExternalInput").ap()
    bi = nc.dram_tensor("bi", [1, _IPC * _G], BF16, kind="ExternalInput").ap()
    # b_up col-major; col 4 is a zero column used as the ACT bias AP
    buc = nc.dram_tensor("buc", [128, 5], F32, kind="ExternalInput").ap()
    on = nc.dram_tensor("on", [1, 512], BF16, kind="ExternalInput").ap()
    # outputs: per pair, both experts' logits for every sample in the segment
    ot = nc.dram_tensor("ot", [NTcols, 2 * _G], FP16, kind="ExternalOutput").ap()
    oi = nc.dram_tensor("oi", [NIcols, 2 * _G], FP16, kind="ExternalOutput").ap()

    otv = ot.rearrange("(j s) g -> s j g", s=Cpt)
    oiv = oi.rearrange("(j s) g -> s j g", s=Cpi)

    with tile.TileContext(nc) as tc:
        with (
            tc.tile_pool(name="persist", bufs=1) as persist,
            tc.tile_pool(name="opool", bufs=6) as opool,
            tc.tile_pool(name="pspool", bufs=8, space="PSUM") as pspool,
        ):
            # ---- input DMAs, ordered so each engine FIFO streams from t=0.
            # Stage-1's critical inputs lead both HWDGE rings (the SWDGE
            # queue is far too slow for them, ~40 GB/s): one combined
            # [featT | W_upT] DMA per ring, then all weight half-chunks
            # back-to-back.
            s1a_sb = persist.tile(
                [128, 2, NTcols + _HID], BF16, name="s1a_sb", tag="s1a_sb"
            )
            s1b_sb = persist.tile([128, 2, NIcols], BF16, name="s1b_sb", tag="s1b_sb")
            buc_sb = persist.tile([128, 5], F32, name="buc_sb", tag="buc_sb")

            w_sb = {}
            for role, wdram, nch in (("t", wt, TCH), ("i", wi, ICH)):
                for ch in range(nch):
                    w_sb[(role, ch)] = persist.tile(
                        [128, _CPE, 4, _G], FP8E3,
                        name=f"w_{role}{ch}", tag=f"w_{role}{ch}",
                    )

            def wdma(role, ch):
                wdram = wt if role == "t" else wi
                wv = wdram[ch].rearrange("p (e c g) -> p e c g", e=_CPE, c=4)
                h = _CPE // 2
                nc.sync.dma_start(out=w_sb[(role, ch)][:, :h], in_=wv[:, :h])
                nc.scalar.dma_start(out=w_sb[(role, ch)][:, h:], in_=wv[:, h:])

            # ring order: stage-1 table inputs, first table weight chunk,
            # index features, then the remaining chunks — so stage-2 table
            # can start the moment stage-1 table finishes. The final index
            # chunk rides the slow-but-idle SWDGE queue (it has ~15us of
            # slack), freeing 0.5 MB of ring bandwidth for earlier chunks.
            nc.sync.dma_start(out=s1a_sb[:, 0], in_=s1a[:, 0])
            nc.scalar.dma_start(out=s1a_sb[:, 1], in_=s1a[:, 1])
            nc.sync.dma_start(out=buc_sb, in_=buc)
            wdma("t", 0)
            nc.sync.dma_start(out=s1b_sb[:, 0], in_=s1b[:, 0])
            nc.scalar.dma_start(out=s1b_sb[:, 1], in_=s1b[:, 1])
            wdma("t", 1)
            for ch in range(ICH):
                wdma("i", ch)

            # PE warmup: dummy matmuls during the initial DMA-latency window
            # pre-ramp the HAM clock gate before real work arrives (the warm
            # tile memset leads the gpsimd FIFO so warmups start early)
            warm = persist.tile([128, 512], BF16, name="warm", tag="warm")
            nc.gpsimd.memset(warm, 0.0)

            # non-critical small inputs + the final weight chunk on SWDGE
            bt_sb = persist.tile([1, _TPC * _G], BF16, name="bt_sb", tag="bt_sb")
            nc.gpsimd.dma_start(out=bt_sb, in_=bt)
            bi_sb = persist.tile([1, _IPC * _G], BF16, name="bi_sb", tag="bi_sb")
            nc.gpsimd.dma_start(out=bi_sb, in_=bi)
            ones = persist.tile([1, 512], BF16, name="ones", tag="ones")
            nc.gpsimd.dma_start(out=ones, in_=on)

            psw = pspool.tile([128, 512], F32, name="ps", tag="ps")
            for _ in range(6):
                nc.tensor.matmul(psw, lhsT=warm[:, :128], rhs=warm, start=True, stop=True)

            # ---- stage 1: xT = relu(W_upT.T @ featT + b_up)
            x_sb = {}
            for role, NC in (("t", NTcols), ("i", NIcols)):
                x_sb[role] = [
                    persist.tile(
                        [128, NC], BF16, name=f"x_{role}{m}", tag=f"x_{role}{m}"
                    )
                    for m in range(4)
                ]

            def stage1(role, NC):
                # interleave m-chunk pairs so consecutive matmuls hit
                # different PSUM banks
                f_src = s1a_sb if role == "t" else s1b_sb
                for n0 in range(0, NC, 512):
                    nw = min(512, NC - n0)
                    for m0 in (0, 2):
                        ps1s = {
                            m: pspool.tile([128, 512], F32, name="ps", tag="ps")
                            for m in (m0, m0 + 1)
                        }
                        for c in range(2):
                            for m in (m0, m0 + 1):
                                nc.tensor.matmul(
                                    ps1s[m][:, :nw],
                                    lhsT=s1a_sb[
                                        :, c, NTcols + m * 128 : NTcols + (m + 1) * 128
                                    ],
                                    rhs=f_src[:, c, n0 : n0 + nw],
                                    start=(c == 0),
                                    stop=(c == 1),
                                )
                        for m in (m0, m0 + 1):
                            # relu+bias on the idle DVE (one fused op) so the
                            # Scalar engine stays free for ring dispatch
                            nc.vector.tensor_scalar(
                                out=x_sb[role][m][:, n0 : n0 + nw],
                                in0=ps1s[m][:, :nw],
                                scalar1=buc_sb[:, m : m + 1],
                                scalar2=0.0,
                                op0=mybir.AluOpType.add,
                                op1=mybir.AluOpType.max,
                            )

            # ---- stage 2: weight chunks of 4 experts = 2 pairs. One pair =>
            # one column segment, 4 fused K-chunk matmuls with fp8 rhs
            # spanning both experts (N=512), one sigmoid (descale folded into
            # ACT scale). Index pairs (M = Cpi <= 64) run BOTH pairs of a
            # chunk concurrently in the two 128x64 column-tile halves of the
            # PE array — two independent moving streams, ~2x throughput.
            itiled = Cpi <= 64
            last = ("i", ICH - 1)

            def stage2(role, ch):
                bsb, ov, C = (
                    (bt_sb, otv, Cpt) if role == "t" else (bi_sb, oiv, Cpi)
                )
                xs = x_sb[role]
                wtile = w_sb[(role, ch)]
                tiled = role == "i" and itiled
                share = tiled and (role, ch) == last
                for s0 in range(0, C, 128):
                    sw = min(128, C - s0)
                    prs = list(range(_CPE // 2))
                    po = {pr: 64 * pr if tiled else 0 for pr in prs}
                    tp = {pr: (0, 64 * pr) if tiled else None for pr in prs}
                    if share:
                        # final chunk: both pairs share one PSUM bank (the
                        # accumulation groups are tracked per partition), the
                        # M=64 bias initializes every partition so ONE sigmoid
                        # covers both pairs — halves the kernel-tail ACT time.
                        # The bank is never reused, so the bias-only rows'
                        # still-open groups are harmless.
                        shared = pspool.tile([128, 512], F32, name="ps", tag="ps")
                        ps2s = {pr: shared for pr in prs}
                        bw = 64
                    else:
                        ps2s = {
                            pr: pspool.tile([128, 512], F32, name="ps", tag="ps")
                            for pr in prs
                        }
                        bw = None
                    # bias first: initializes the PSUM zero region for
                    # accumulation; pairs interleaved (bank-alternating in
                    # the untiled case, array-concurrent in the tiled case)
                    for pr in prs:
                        k0 = 2 * (ch * (_CPE // 2) + pr)
                        nc.tensor.matmul(
                            ps2s[pr][po[pr] : po[pr] + (bw or sw), :],
                            lhsT=ones[:, : (bw or sw)],
                            rhs=bsb[:, k0 * _G : (k0 + 2) * _G],
                            start=True,
                            stop=False,
                            tile_position=tp[pr],
                        )
                    for c in range(4):
                        for pr in prs:
                            j = ch * (_CPE // 2) + pr
                            nc.tensor.matmul(
                                ps2s[pr][po[pr] : po[pr] + sw, :],
                                lhsT=xs[c][:, j * C + s0 : j * C + s0 + sw],
                                rhs=wtile[:, 2 * pr : 2 * pr + 2, c, :],
                                start=False,
                                stop=(c == 3),
                                tile_position=tp[pr],
                            )
                    o_sb = opool.tile(
                        [128, 1 if share else len(prs), 2 * _G],
                        FP16, name="o_sb", tag="o_sb",
                    )
                    if share:
                        nc.scalar.activation(
                            out=o_sb[:, 0],
                            in_=ps2s[0],
                            func=AF.Sigmoid,
                            bias=buc_sb[:, 4:5],
                            scale=1.0 / _WSCALE,
                        )
                    else:
                        for pr in prs:
                            nc.scalar.activation(
                                out=o_sb[po[pr] : po[pr] + sw, pr],
                                in_=ps2s[pr][po[pr] : po[pr] + sw, :],
                                func=AF.Sigmoid,
                                bias=buc_sb[po[pr] : po[pr] + sw, 4:5],
                                scale=1.0 / _WSCALE,
                            )
                    j0 = ch * len(prs)
                    if (role, ch) == last:
                        # kernel tail: final outputs on the low-latency
                        # HWDGE rings, split across both
                        c0 = 0
                        nc.sync.dma_start(
                            out=ov[s0 : s0 + sw, j0 : j0 + 1, :],
                            in_=o_sb[po[0] : po[0] + sw, c0 : c0 + 1],
                        )
                        nc.scalar.dma_start(
                            out=ov[s0 : s0 + sw, j0 + 1 : j0 + 2, :],
                            in_=o_sb[po[1] : po[1] + sw, c0 : c0 + 1]
                            if share
                            else o_sb[po[1] : po[1] + sw, 1:2],
                        )
                    elif tiled:
                        # pairs live at different partition offsets: one
                        # DMA per pair on the idle sync ring
                        for pr in prs:
                            nc.sync.dma_start(
                                out=ov[s0 : s0 + sw, j0 + pr : j0 + pr + 1, :],
                                in_=o_sb[po[pr] : po[pr] + sw, pr : pr + 1],
                            )
                    else:
                        nc.sync.dma_start(
                            out=ov[s0 : s0 + sw, j0 : j0 + len(prs), :],
                            in_=o_sb[:sw],
                        )

            # emission order = Tensor FIFO order: table stage-1, first table
            # chunk (its weights arrive right behind the stage-1 inputs),
            # index stage-1 (fills PE while more weights stream), the rest.
            stage1("t", NTcols)
            stage2("t", 0)
            stage1("i", NIcols)
            stage2("t", 1)
            for ch in range(ICH):
                stage2("i", ch)

    nc.compile()
    return nc


def _get_nc(Cpt, Cpi):
    key = (Cpt, Cpi)
    if key not in _nc_cache:
        _nc_cache[key] = _build(Cpt, Cpi)
    return _nc_cache[key]


def _pack_weights(W, nexp):
    """[nexp, G, HID] -> [nexp/_CPE, 128, _CPE*4*G] partition-major fp8 chunks."""
    nch = nexp // _CPE
    A = (W * _WSCALE).reshape(nch, _CPE, _G, 4, 128)      # [ch, e, g, c, p]
    A = np.ascontiguousarray(A.transpose(0, 4, 1, 3, 2))  # [ch, p, e, c, g]
    return A.reshape(nch, 128, _CPE * 4 * _G).astype(ml_dtypes.float8_e3m4)


def _route(ids, mask, n_experts):
    """Per-PAIR sample lists with load-balanced pairing: the heaviest-loaded
    expert is paired with the lightest, so the max pair load (which sets the
    uniform capacity = fa/output padding) stays near 2x the mean instead of
    max+max. Returns (perm, pair_samples, pair_parity); pair j owns experts
    perm[2j], perm[2j+1]."""
    per_expert = [np.flatnonzero((ids == e) & mask) for e in range(n_experts)]
    order = np.argsort([-len(s) for s in per_expert], kind="stable")
    perm = np.empty(n_experts, np.int64)
    for j in range(n_experts // 2):
        perm[2 * j] = order[j]
        perm[2 * j + 1] = order[n_experts - 1 - j]
    samples, parity = [], []
    for j in range(n_experts // 2):
        s0 = per_expert[perm[2 * j]]
        s1 = per_expert[perm[2 * j + 1]]
        samples.append(np.concatenate([s0, s1]))
        parity.append(np.concatenate([np.zeros(len(s0), np.int64),
                                      np.ones(len(s1), np.int64)]))
    return perm, samples, parity


def kernel(
    feature,
    table_ids,
    index_ids,
    table_mask,
    index_mask,
    W_up,
    b_up,
    W_table,
    b_table,
    W_index,
    b_index,
):
    global LAST_RESULTS
    from concourse.bass_utils import run_bass_kernel_spmd

    feature = np.ascontiguousarray(np.asarray(feature), dtype=np.float32)
    table_ids = np.asarray(table_ids).astype(np.int64)
    index_ids = np.asarray(index_ids).astype(np.int64)
    table_mask = np.asarray(table_mask).astype(bool)
    index_mask = np.asarray(index_mask).astype(bool)
    W_up = np.asarray(W_up, dtype=np.float32)
    b_up = np.asarray(b_up, dtype=np.float32)
    W_table = np.asarray(W_table, dtype=np.float32)
    b_table = np.asarray(b_table, dtype=np.float32)
    W_index = np.asarray(W_index, dtype=np.float32)
    b_index = np.asarray(b_index, dtype=np.float32)

    B = feature.shape[0]

    perm_t, smp_t, par_t = _route(table_ids, table_mask, _N_TABLES)
    perm_i, smp_i, par_i = _route(index_ids, index_mask, _N_INDEXES)
    # Uniform per-pair capacity so all 8 cores run one identical program.
    Cpt = max(8, -(-max(len(s) for s in smp_t) // 8) * 8)
    Cpi = max(8, -(-max(len(s) for s in smp_i) // 8) * 8)

    nc = _get_nc(Cpt, Cpi)

    TP = _TPC // 2
    IP = _IPC // 2
    W_upT = np.ascontiguousarray(W_up.T).astype(ml_dtypes.bfloat16)
    buc = np.zeros((128, 5), np.float32)
    buc[:, :4] = b_up.reshape(4, 128).T
    ones = np.ones((1, 512), np.float32).astype(ml_dtypes.bfloat16)

    NA = TP * Cpt + IP * Cpi
    in_maps = []
    for c in range(_N_CORES):
        ts = perm_t[c * _TPC : (c + 1) * _TPC]
        is_ = perm_i[c * _IPC : (c + 1) * _IPC]
        # [featT_table | W_upT] then [featT_index], long contiguous rows
        s1a_c = np.zeros((128, 2, TP * Cpt + _HID), ml_dtypes.bfloat16)
        s1b_c = np.zeros((128, 2, IP * Cpi), ml_dtypes.bfloat16)
        for j in range(TP):
            s = smp_t[c * TP + j]
            if len(s):
                s1a_c[:, :, j * Cpt : j * Cpt + len(s)] = (
                    feature[s].T.reshape(2, 128, len(s)).transpose(1, 0, 2)
                )
        for j in range(IP):
            s = smp_i[c * IP + j]
            if len(s):
                s1b_c[:, :, j * Cpi : j * Cpi + len(s)] = (
                    feature[s].T.reshape(2, 128, len(s)).transpose(1, 0, 2)
                )
        s1a_c[:, :, TP * Cpt :] = W_upT.reshape(2, 128, _HID).transpose(1, 0, 2)
        in_maps.append(
            {
                "s1a": s1a_c,
                "s1b": s1b_c,
                "wt": _pack_weights(W_table[ts], _TPC),
                "wi": _pack_weights(W_index[is_], _IPC),
                "bt": (b_table[ts].reshape(1, -1) * np.float32(_WSCALE)).astype(
                    ml_dtypes.bfloat16
                ),
                "bi": (b_index[is_].reshape(1, -1) * np.float32(_WSCALE)).astype(
                    ml_dtypes.bfloat16
                ),
                "buc": buc,
                "on": ones,
            }
        )

    res = run_bass_kernel_spmd(
        nc, in_maps, core_ids=list(range(_N_CORES)), trace=TRACE
    )
    LAST_RESULTS = res

    out = np.zeros((2, B, _G), np.float32)
    for c in range(_N_CORES):
        rt = res.results[c]["ot"].astype(np.float32)
        ri = res.results[c]["oi"].astype(np.float32)
        for j in range(TP):
            s = smp_t[c * TP + j]
            if len(s):
                rows = rt[j * Cpt : j * Cpt + len(s)].reshape(len(s), 2, _G)
                out[0, s, :] = rows[np.arange(len(s)), par_t[c * TP + j], :]
        for j in range(IP):
            s = smp_i[c * IP + j]
            if len(s):
                rows = ri[j * Cpi : j * Cpi + len(s)].reshape(len(s), 2, _G)
                out[1, s, :] = rows[np.arange(len(s)), par_i[c * IP + j], :]
    return out
